# revision 3
# baseline (speedup 1.0000x reference)
"""Trainium2 Bass kernel for nn_InvariantHeadviaTP.

Reference computation (after dead-code elimination -- y1/y2/gates are never
used by the output):
    x0   = node_vec[:, :128]                  # [N, 128]
    a    = node_embedding                     # [N, 16]
    s0   = einsum('ni,na,iak->nk', x0, a, W1_l0[:, :, :128]) / sqrt(2048) + b1[:128]
    scal = silu(s0)                           # [N, 128]
    mid  = einsum('ni,na,iak->nk', scal, a, W2) / sqrt(2048) + b2   # [N, 16]
    h    = silu(mid @ W3 / 4 + b3)            # [N, 16]
    out  = h @ W4 / 4 + b4                    # [N, 1]

Strategy: data-parallel over 8 cores (2048 nodes each). Per core, work in a
transposed layout: features on SBUF partitions, nodes on the free dim.
The bilinear contractions over c=(a,i) [16*128=2048] are done as 16
PSUM-accumulated matmuls with bf16 operands:
    s0T[k, n] = sum_a sum_i W[i,a,k] * (x0T[i,n] * aT[a,n])
The inner operand U_a[i,n] = x0T[i,n]*aT[a,n] is built on the vector engine
as a tensor_tensor multiply against "Arep_a" = row a of aT broadcast across
128 partitions; Arep is produced on the tensor engine via a one-hot selector
matmul (out = sel_a.T @ aT = broadcast).
"""

import numpy as np
import ml_dtypes
from contextlib import ExitStack

import concourse.bass as bass
import concourse.mybir as mybir
import concourse.tile as tile
from concourse import bacc
from concourse.bass import ts
from concourse.bass_utils import run_bass_kernel_spmd

N_CORES = 8
N_FULL = 16384
NSH = N_FULL // N_CORES          # 2048 nodes per core
A = 16                           # attr dim
M0 = 128                         # MUL0 (scalar channels)
FREE = 512                       # node tile (free dim) per inner step
SCALE = 1.0 / np.sqrt(M0 * A)    # path normalization of both fctp einsums
BF16 = ml_dtypes.bfloat16

AF = mybir.ActivationFunctionType
F32 = mybir.dt.float32
DBF16 = mybir.dt.bfloat16


def build_nc(nsh: int = NSH, free: int = FREE, num_devices: int = N_CORES):
    nc = bacc.Bacc(
        "TRN2",
        target_bir_lowering=False,
        debug=False,
        enable_asserts=False,
        num_devices=num_devices,
    )

    x0t = nc.dram_tensor("x0t", [M0, nsh], DBF16, kind="ExternalInput").ap()
    at = nc.dram_tensor("at", [A, nsh], DBF16, kind="ExternalInput").ap()
    w0 = nc.dram_tensor("w0", [M0, A * M0], DBF16, kind="ExternalInput").ap()
    w2 = nc.dram_tensor("w2", [M0, A * A], DBF16, kind="ExternalInput").ap()
    w3 = nc.dram_tensor("w3", [A, A], DBF16, kind="ExternalInput").ap()
    w4 = nc.dram_tensor("w4", [A, 1], DBF16, kind="ExternalInput").ap()
    sel = nc.dram_tensor("sel", [A, A * M0], DBF16, kind="ExternalInput").ap()
    b1 = nc.dram_tensor("b1", [M0, 1], F32, kind="ExternalInput").ap()
    b2 = nc.dram_tensor("b2", [A, 1], F32, kind="ExternalInput").ap()
    b3 = nc.dram_tensor("b3", [A, 1], F32, kind="ExternalInput").ap()
    b4 = nc.dram_tensor("b4", [1, 1], F32, kind="ExternalInput").ap()
    outt = nc.dram_tensor("outt", [1, nsh], F32, kind="ExternalOutput").ap()

    n_tiles = nsh // free

    with tile.TileContext(nc) as tc, ExitStack() as ctx:
        consts = ctx.enter_context(tc.tile_pool(name="consts", bufs=1))

        x0t_sb = consts.tile([M0, nsh], DBF16)
        nc.sync.dma_start(x0t_sb[:], x0t)
        at_sb = consts.tile([A, nsh], DBF16)
        nc.sync.dma_start(at_sb[:], at)
        w0_sb = consts.tile([M0, A * M0], DBF16)
        nc.sync.dma_start(w0_sb[:], w0)
        w2_sb = consts.tile([M0, A * A], DBF16)
        nc.sync.dma_start(w2_sb[:], w2)
        w3_sb = consts.tile([A, A], DBF16)
        nc.sync.dma_start(w3_sb[:], w3)
        w4_sb = consts.tile([A, 1], DBF16)
        nc.sync.dma_start(w4_sb[:], w4)
        sel_sb = consts.tile([A, A * M0], DBF16)
        nc.sync.dma_start(sel_sb[:], sel)
        b1_sb = consts.tile([M0, 1], F32)
        nc.sync.dma_start(b1_sb[:], b1)
        b2_sb = consts.tile([A, 1], F32)
        nc.sync.dma_start(b2_sb[:], b2)
        b3_sb = consts.tile([A, 1], F32)
        nc.sync.dma_start(b3_sb[:], b3)
        b4_sb = consts.tile([1, 1], F32)
        nc.sync.dma_start(b4_sb[:], b4)

        arep_pool = ctx.enter_context(tc.tile_pool(name="arep", bufs=2))
        u_pool = ctx.enter_context(tc.tile_pool(name="u", bufs=3))
        s_pool = ctx.enter_context(tc.tile_pool(name="s", bufs=2))
        o_pool = ctx.enter_context(tc.tile_pool(name="o", bufs=2))
        ps_arep = ctx.enter_context(tc.tile_pool(name="ps_arep", bufs=2, space="PSUM"))
        ps_s0 = ctx.enter_context(tc.tile_pool(name="ps_s0", bufs=2, space="PSUM"))
        ps_mid = ctx.enter_context(tc.tile_pool(name="ps_mid", bufs=2, space="PSUM"))
        ps_mlp = ctx.enter_context(tc.tile_pool(name="ps_mlp", bufs=1, space="PSUM"))

        for t in range(n_tiles):
            sl = ts(t, free)

            # Arep_a[p, n] = aT[a, n] for all p, via one-hot selector matmul.
            arep = arep_pool.tile([M0, A * free], DBF16)
            for a in range(A):
                arep_ps = ps_arep.tile([M0, free], F32)
                nc.tensor.matmul(
                    arep_ps[:], sel_sb[:, ts(a, M0)], at_sb[:, sl],
                    start=True, stop=True,
                )
                nc.scalar.copy(arep[:, ts(a, free)], arep_ps[:])

            # s0T accumulation over the 16 a-chunks of c=(a,i).
            s0_ps = ps_s0.tile([M0, free], F32)
            for a in range(A):
                u0 = u_pool.tile([M0, free], DBF16, tag="u0")
                nc.vector.tensor_mul(u0[:], x0t_sb[:, sl], arep[:, ts(a, free)])
                nc.tensor.matmul(
                    s0_ps[:], w0_sb[:, ts(a, M0)], u0[:],
                    start=(a == 0), stop=(a == A - 1),
                )

            # silu(s0 + b1) = (s0+b1) * sigmoid(s0+b1); CoreSim has no Silu LUT.
            s_pre = s_pool.tile([M0, free], DBF16, tag="s_pre")
            nc.scalar.activation(s_pre[:], s0_ps[:], AF.Identity, bias=b1_sb[:])
            s_sig = s_pool.tile([M0, free], DBF16, tag="s_sig")
            nc.scalar.activation(s_sig[:], s0_ps[:], AF.Sigmoid, bias=b1_sb[:])
            scal = s_pool.tile([M0, free], DBF16, tag="scal")
            nc.vector.tensor_mul(scal[:], s_pre[:], s_sig[:])

            # midT accumulation.
            mid_ps = ps_mid.tile([A, free], F32)
            for a in range(A):
                u3 = u_pool.tile([M0, free], DBF16, tag="u3")
                nc.vector.tensor_mul(u3[:], scal[:], arep[:, ts(a, free)])
                nc.tensor.matmul(
                    mid_ps[:], w2_sb[:, ts(a, A)], u3[:],
                    start=(a == 0), stop=(a == A - 1),
                )

            midb = s_pool.tile([A, free], DBF16, tag="midb")
            nc.scalar.activation(midb[:], mid_ps[:], AF.Identity, bias=b2_sb[:])

            h_ps = ps_mlp.tile([A, free], F32, tag="h")
            nc.tensor.matmul(h_ps[:], w3_sb[:], midb[:], start=True, stop=True)
            h_pre = s_pool.tile([A, free], DBF16, tag="h_pre")
            nc.scalar.activation(h_pre[:], h_ps[:], AF.Identity, bias=b3_sb[:])
            h_sig = s_pool.tile([A, free], DBF16, tag="h_sig")
            nc.scalar.activation(h_sig[:], h_ps[:], AF.Sigmoid, bias=b3_sb[:])
            hb = s_pool.tile([A, free], DBF16, tag="hb")
            nc.vector.tensor_mul(hb[:], h_pre[:], h_sig[:])

            out_ps = ps_mlp.tile([1, free], F32, tag="out")
            nc.tensor.matmul(out_ps[:], w4_sb[:], hb[:], start=True, stop=True)
            ob = o_pool.tile([1, free], F32)
            nc.scalar.activation(ob[:], out_ps[:], AF.Identity, bias=b4_sb[:])
            nc.sync.dma_start(outt[:, sl], ob[:])

    nc.compile()
    return nc


def prep_host(inputs: dict, nsh: int = NSH, n_cores: int = N_CORES):
    """Host-side prep: slice/transpose/cast inputs, build per-core in_maps."""
    node_vec = np.asarray(inputs["node_vec"], dtype=np.float32)
    node_embedding = np.asarray(inputs["node_embedding"], dtype=np.float32)
    W1_l0 = np.asarray(inputs["W1_l0"], dtype=np.float32)
    b1 = np.asarray(inputs["b1"], dtype=np.float32)
    W2 = np.asarray(inputs["W2"], dtype=np.float32)
    b2 = np.asarray(inputs["b2"], dtype=np.float32)
    W3 = np.asarray(inputs["W3"], dtype=np.float32)
    b3 = np.asarray(inputs["b3"], dtype=np.float32)
    W4 = np.asarray(inputs["W4"], dtype=np.float32)
    b4 = np.asarray(inputs["b4"], dtype=np.float32)

    x0T = np.ascontiguousarray(node_vec[:, :M0].T).astype(BF16)      # [128, N]
    aT = np.ascontiguousarray(node_embedding.T).astype(BF16)         # [16, N]

    w0h = (W1_l0[:, :, :M0] * SCALE).reshape(M0, A * M0).astype(BF16)
    w2h = (W2 * SCALE).reshape(M0, A * A).astype(BF16)
    w3h = (W3 / np.sqrt(A)).astype(BF16)
    w4h = (W4 / np.sqrt(A)).astype(BF16)

    selh = np.zeros((A, A * M0), dtype=BF16)
    for a in range(A):
        selh[a, a * M0:(a + 1) * M0] = 1.0

    shared = {
        "w0": w0h, "w2": w2h, "w3": w3h, "w4": w4h, "sel": selh,
        "b1": np.ascontiguousarray(b1[:M0].reshape(M0, 1)),
        "b2": np.ascontiguousarray(b2.reshape(A, 1)),
        "b3": np.ascontiguousarray(b3.reshape(A, 1)),
        "b4": np.ascontiguousarray(b4.reshape(1, 1)),
    }
    in_maps = []
    for c in range(n_cores):
        sl = slice(c * nsh, (c + 1) * nsh)
        in_maps.append({
            "x0t": np.ascontiguousarray(x0T[:, sl]),
            "at": np.ascontiguousarray(aT[:, sl]),
            **shared,
        })
    return in_maps


_NC_CACHE = {}


def _get_nc():
    if "nc" not in _NC_CACHE:
        _NC_CACHE["nc"] = build_nc()
    return _NC_CACHE["nc"]


def kernel_with_results(trace: bool = False, **inputs):
    nc = _get_nc()
    in_maps = prep_host(inputs)
    res = run_bass_kernel_spmd(
        nc, in_maps, core_ids=list(range(N_CORES)), trace=trace,
    )
    out = np.empty((N_FULL, 1), dtype=np.float32)
    for c in range(N_CORES):
        out[c * NSH:(c + 1) * NSH, 0] = res.results[c]["outt"][0]
    return out, res


def kernel(**inputs) -> np.ndarray:
    out, _ = kernel_with_results(trace=False, **inputs)
    return out


# revision 10
# speedup vs baseline: 1.0388x; 1.0388x over previous
"""Trainium2 Bass kernel for nn_InvariantHeadviaTP.

Reference computation (after dead-code elimination -- y1/y2/gates are never
used by the output):
    x0   = node_vec[:, :128]                  # [N, 128]
    a    = node_embedding                     # [N, 16]
    s0   = einsum('ni,na,iak->nk', x0, a, W1_l0[:, :, :128]) / sqrt(2048) + b1[:128]
    scal = silu(s0)                           # [N, 128]
    mid  = einsum('ni,na,iak->nk', scal, a, W2) / sqrt(2048) + b2   # [N, 16]
    h    = silu(mid @ W3 / 4 + b3)            # [N, 16]
    out  = h @ W4 / 4 + b4                    # [N, 1]

Strategy: data-parallel over 8 cores (2048 nodes each). Per core, work in a
transposed layout: features on SBUF partitions, nodes on the free dim.
The bilinear contractions over c=(a,i) [16*128=2048] are done as 16
PSUM-accumulated matmuls with bf16 operands:
    s0T[k, n] = sum_a sum_i W[i,a,k] * (x0T[i,n] * aT[a,n])
The inner operand U_a[i,n] = x0T[i,n]*aT[a,n] is built on the vector engine
as a tensor_tensor multiply against "Arep_a" = row a of aT broadcast across
128 partitions; Arep is produced on the tensor engine via a one-hot selector
matmul (out = sel_a.T @ aT = broadcast).
"""

import numpy as np
import ml_dtypes
from contextlib import ExitStack

import concourse.bass as bass
import concourse.mybir as mybir
import concourse.tile as tile
from concourse import bacc
from concourse.bass import ts
from concourse.bass_utils import run_bass_kernel_spmd

N_CORES = 8
N_FULL = 16384
NSH = N_FULL // N_CORES          # 2048 nodes per core
A = 16                           # attr dim
M0 = 128                         # MUL0 (scalar channels)
FREE = 512                       # node tile (free dim) per inner step
SCALE = 1.0 / np.sqrt(M0 * A)    # path normalization of both fctp einsums
BF16 = ml_dtypes.bfloat16

AF = mybir.ActivationFunctionType
F32 = mybir.dt.float32
DBF16 = mybir.dt.bfloat16


def build_nc(nsh: int = NSH, free: int = FREE, num_devices: int = N_CORES):
    nc = bacc.Bacc(
        "TRN2",
        target_bir_lowering=False,
        debug=False,
        enable_asserts=False,
        num_devices=num_devices,
    )

    x0t = nc.dram_tensor("x0t", [M0, nsh], DBF16, kind="ExternalInput").ap()
    at = nc.dram_tensor("at", [A, nsh], DBF16, kind="ExternalInput").ap()
    w0 = nc.dram_tensor("w0", [M0, A * M0], DBF16, kind="ExternalInput").ap()
    w2 = nc.dram_tensor("w2", [M0, A * A], DBF16, kind="ExternalInput").ap()
    w3 = nc.dram_tensor("w3", [A, A], DBF16, kind="ExternalInput").ap()
    w4 = nc.dram_tensor("w4", [A, 1], DBF16, kind="ExternalInput").ap()
    b1 = nc.dram_tensor("b1", [M0, 1], F32, kind="ExternalInput").ap()
    b2 = nc.dram_tensor("b2", [A, 1], F32, kind="ExternalInput").ap()
    b3 = nc.dram_tensor("b3", [A, 1], F32, kind="ExternalInput").ap()
    b4 = nc.dram_tensor("b4", [1, 1], F32, kind="ExternalInput").ap()
    outt = nc.dram_tensor("outt", [1, nsh], F32, kind="ExternalOutput").ap()

    n_tiles = nsh // free

    with tile.TileContext(nc) as tc, ExitStack() as ctx:
        consts = ctx.enter_context(tc.tile_pool(name="consts", bufs=1))

        x0t_sb = consts.tile([M0, nsh], DBF16)
        nc.sync.dma_start(x0t_sb[:], x0t)
        at_sb = consts.tile([A, nsh], DBF16)
        nc.sync.dma_start(at_sb[:], at)
        # per-row copies at partition 0 (partition_broadcast src requirement)
        at_rows = []
        for a in range(A):
            at_row = consts.tile([1, nsh], DBF16, name=f"at_row{a}")
            nc.sync.dma_start(at_row[:], at[a:a + 1, :])
            at_rows.append(at_row)
        w0_sb = consts.tile([M0, A * M0], DBF16)
        nc.sync.dma_start(w0_sb[:], w0)
        w2_sb = consts.tile([M0, A * A], DBF16)
        nc.sync.dma_start(w2_sb[:], w2)
        w3_sb = consts.tile([A, A], DBF16)
        nc.sync.dma_start(w3_sb[:], w3)
        w4_sb = consts.tile([A, 1], DBF16)
        nc.sync.dma_start(w4_sb[:], w4)
        b1_sb = consts.tile([M0, 1], F32)
        nc.sync.dma_start(b1_sb[:], b1)
        b2_sb = consts.tile([A, 1], F32)
        nc.sync.dma_start(b2_sb[:], b2)
        b3_sb = consts.tile([A, 1], F32)
        nc.sync.dma_start(b3_sb[:], b3)
        b4_sb = consts.tile([1, 1], F32)
        nc.sync.dma_start(b4_sb[:], b4)

        arep_pool = ctx.enter_context(tc.tile_pool(name="arep", bufs=2))
        u_pool = ctx.enter_context(tc.tile_pool(name="u", bufs=3))
        s_pool = ctx.enter_context(tc.tile_pool(name="s", bufs=2))
        o_pool = ctx.enter_context(tc.tile_pool(name="o", bufs=2))
        ps_s0 = ctx.enter_context(tc.tile_pool(name="ps_s0", bufs=2, space="PSUM"))
        ps_mid = ctx.enter_context(tc.tile_pool(name="ps_mid", bufs=2, space="PSUM"))
        ps_mlp = ctx.enter_context(tc.tile_pool(name="ps_mlp", bufs=1, space="PSUM"))

        for t in range(n_tiles):
            sl = ts(t, free)

            # Arep_a[p, n] = aT[a, n] for all p, via GPSIMD partition broadcast.
            arep = arep_pool.tile([M0, A * free], DBF16)
            for a in range(A):
                nc.gpsimd.partition_broadcast(
                    arep[:, ts(a, free)], at_rows[a][:, sl]
                )

            # s0T accumulation over the 16 a-chunks of c=(a,i).
            s0_ps = ps_s0.tile([M0, free], F32)
            for a in range(A):
                u0 = u_pool.tile([M0, free], DBF16, tag="u0")
                nc.vector.tensor_mul(u0[:], x0t_sb[:, sl], arep[:, ts(a, free)])
                nc.tensor.matmul(
                    s0_ps[:], w0_sb[:, ts(a, M0)], u0[:],
                    start=(a == 0), stop=(a == A - 1),
                )

            # silu(s0 + b1) = (s0+b1) * sigmoid(s0+b1); CoreSim has no Silu LUT.
            s_pre = s_pool.tile([M0, free], DBF16, tag="s_pre")
            nc.scalar.activation(s_pre[:], s0_ps[:], AF.Identity, bias=b1_sb[:])
            s_sig = s_pool.tile([M0, free], DBF16, tag="s_sig")
            nc.scalar.activation(s_sig[:], s0_ps[:], AF.Sigmoid, bias=b1_sb[:])
            scal = s_pool.tile([M0, free], DBF16, tag="scal")
            nc.vector.tensor_mul(scal[:], s_pre[:], s_sig[:])

            # midT accumulation.
            mid_ps = ps_mid.tile([A, free], F32)
            for a in range(A):
                u3 = u_pool.tile([M0, free], DBF16, tag="u3")
                nc.vector.tensor_mul(u3[:], scal[:], arep[:, ts(a, free)])
                nc.tensor.matmul(
                    mid_ps[:], w2_sb[:, ts(a, A)], u3[:],
                    start=(a == 0), stop=(a == A - 1),
                )

            midb = s_pool.tile([A, free], DBF16, tag="midb")
            nc.scalar.activation(midb[:], mid_ps[:], AF.Identity, bias=b2_sb[:])

            h_ps = ps_mlp.tile([A, free], F32, tag="h")
            nc.tensor.matmul(h_ps[:], w3_sb[:], midb[:], start=True, stop=True)
            h_pre = s_pool.tile([A, free], DBF16, tag="h_pre")
            nc.scalar.activation(h_pre[:], h_ps[:], AF.Identity, bias=b3_sb[:])
            h_sig = s_pool.tile([A, free], DBF16, tag="h_sig")
            nc.scalar.activation(h_sig[:], h_ps[:], AF.Sigmoid, bias=b3_sb[:])
            hb = s_pool.tile([A, free], DBF16, tag="hb")
            nc.vector.tensor_mul(hb[:], h_pre[:], h_sig[:])

            out_ps = ps_mlp.tile([1, free], F32, tag="out")
            nc.tensor.matmul(out_ps[:], w4_sb[:], hb[:], start=True, stop=True)
            ob = o_pool.tile([1, free], F32)
            nc.scalar.activation(ob[:], out_ps[:], AF.Identity, bias=b4_sb[:])
            nc.sync.dma_start(outt[:, sl], ob[:])

    nc.compile()
    return nc


def prep_host(inputs: dict, nsh: int = NSH, n_cores: int = N_CORES):
    """Host-side prep: slice/transpose/cast inputs, build per-core in_maps."""
    node_vec = np.asarray(inputs["node_vec"], dtype=np.float32)
    node_embedding = np.asarray(inputs["node_embedding"], dtype=np.float32)
    W1_l0 = np.asarray(inputs["W1_l0"], dtype=np.float32)
    b1 = np.asarray(inputs["b1"], dtype=np.float32)
    W2 = np.asarray(inputs["W2"], dtype=np.float32)
    b2 = np.asarray(inputs["b2"], dtype=np.float32)
    W3 = np.asarray(inputs["W3"], dtype=np.float32)
    b3 = np.asarray(inputs["b3"], dtype=np.float32)
    W4 = np.asarray(inputs["W4"], dtype=np.float32)
    b4 = np.asarray(inputs["b4"], dtype=np.float32)

    x0T = np.ascontiguousarray(node_vec[:, :M0].T).astype(BF16)      # [128, N]
    aT = np.ascontiguousarray(node_embedding.T).astype(BF16)         # [16, N]

    w0h = (W1_l0[:, :, :M0] * SCALE).reshape(M0, A * M0).astype(BF16)
    w2h = (W2 * SCALE).reshape(M0, A * A).astype(BF16)
    w3h = (W3 / np.sqrt(A)).astype(BF16)
    w4h = (W4 / np.sqrt(A)).astype(BF16)

    shared = {
        "w0": w0h, "w2": w2h, "w3": w3h, "w4": w4h,
        "b1": np.ascontiguousarray(b1[:M0].reshape(M0, 1)),
        "b2": np.ascontiguousarray(b2.reshape(A, 1)),
        "b3": np.ascontiguousarray(b3.reshape(A, 1)),
        "b4": np.ascontiguousarray(b4.reshape(1, 1)),
    }
    in_maps = []
    for c in range(n_cores):
        sl = slice(c * nsh, (c + 1) * nsh)
        in_maps.append({
            "x0t": np.ascontiguousarray(x0T[:, sl]),
            "at": np.ascontiguousarray(aT[:, sl]),
            **shared,
        })
    return in_maps


_NC_CACHE = {}


def _get_nc():
    if "nc" not in _NC_CACHE:
        _NC_CACHE["nc"] = build_nc()
    return _NC_CACHE["nc"]


def kernel_with_results(trace: bool = False, **inputs):
    nc = _get_nc()
    in_maps = prep_host(inputs)
    res = run_bass_kernel_spmd(
        nc, in_maps, core_ids=list(range(N_CORES)), trace=trace,
    )
    out = np.empty((N_FULL, 1), dtype=np.float32)
    for c in range(N_CORES):
        out[c * NSH:(c + 1) * NSH, 0] = res.results[c]["outt"][0]
    return out, res


def kernel(**inputs) -> np.ndarray:
    out, _ = kernel_with_results(trace=False, **inputs)
    return out
